# revision 14
# baseline (speedup 1.0000x reference)
"""Trainium2 Bass kernel for nn_AttnBlock (GroupNorm + single-head self-attention + residual).

Full-input contract: kernel(**inputs) takes the unsharded inputs from
setup_inputs() and returns the full [2, 512, 64, 64] output.

Sharding: 8 cores = 2 batches x 4 query-token chunks of 1024 tokens.
Each core receives its batch's x with tokens rotated so its query chunk
is columns 0:1024 (attention / groupnorm are permutation-invariant over
tokens).  Each core computes GroupNorm + full K / V-transpose for its
batch (duplicated across the 4 cores of a batch -- no collectives), and
the attention rows + output projection + residual for its own 1024
queries.

Attention is computed in transposed-score form to avoid all on-chip
transposes:
    sT[m, n]   = sum_c k[c, m] q[c, n]          (lhsT = k, rhs = q)
    vT[m, c]   = sum_ci h[ci, m] wvT[ci, c]     (lhsT = h, rhs = wvT)
    raw[c, n]  = sum_m vT[m, c] expT[m, n]      (lhsT = vT, rhs = expT)
    out[c, n]  = raw[c, n] / denom[n]
The softmax max-subtraction is dropped (scores are O(1) here; exp stays
comfortably in fp32 range and softmax is shift-invariant).  bv folds
into the output-projection bias (softmax rows sum to 1); bk is applied
normally.
"""

import numpy as np

import concourse.bass as bass
import concourse.bacc as bacc
import concourse.mybir as mybir
import concourse.tile as tile
from concourse.bass_utils import run_bass_kernel_spmd

AF = mybir.ActivationFunctionType
ALU = mybir.AluOpType

# Problem shape (hardcoded per contract).
B, C, H, W = 2, 512, 64, 64
HW = H * W            # 4096 tokens
GROUPS, GS = 32, 16   # 16 channels per group
EPS = 1e-6
P = 128               # partitions
CT = C // P           # 4 channel tiles
NQ = HW // 4          # 1024 query tokens per core
NT = NQ // 512        # 2 n-tiles of 512
MT = HW // P          # 32 m-tiles of 128

MODE = "f32r"         # "f32r" | "bf16" matmul operand mode

F32 = mybir.dt.float32
F32R = mybir.dt.float32r
BF16 = mybir.dt.bfloat16
DTW = F32R if MODE == "f32r" else BF16      # weight dram/tile dtype
DTX = F32R if MODE == "f32r" else F32        # x dram/tile dtype (in-place h)
DTT = F32 if MODE == "f32r" else BF16       # mm-feeding computed-tile dtype


def _wc(ap):
    """Cast an output AP that feeds a matmul (producer must round)."""
    return ap.bitcast(F32R) if MODE == "f32r" else ap


def _rc(ap):
    """Cast a matmul operand AP (f32-stored tiles read as f32r)."""
    return ap.bitcast(F32R) if MODE == "f32r" else ap


def _xf(ap):
    """f32 view of an x tile (typed f32r in f32r mode) for non-matmul readers."""
    return ap.bitcast(F32) if MODE == "f32r" else ap


def build_nc():
    nc = bacc.Bacc()

    x_d = nc.dram_tensor("x", [C, HW], DTX, kind="ExternalInput")
    wq_d = nc.dram_tensor("wqt", [C, C], DTW, kind="ExternalInput")
    wk_d = nc.dram_tensor("wkp", [C, C], DTW, kind="ExternalInput")
    wv_d = nc.dram_tensor("wvt", [C, C], DTW, kind="ExternalInput")
    wo_d = nc.dram_tensor("wot", [C, C], DTW, kind="ExternalInput")
    # [128, 4] per-partition vectors (col = channel-tile index)
    bq_d = nc.dram_tensor("bq2", [P, CT], F32, kind="ExternalInput")
    bo_d = nc.dram_tensor("bo2", [P, CT], F32, kind="ExternalInput")
    gns_d = nc.dram_tensor("gns", [P, CT], F32, kind="ExternalInput")
    gnb_d = nc.dram_tensor("gnb", [P, CT], F32, kind="ExternalInput")
    g1_d = nc.dram_tensor("g1", [P, 8], F32, kind="ExternalInput")
    g2_d = nc.dram_tensor("g2", [8, P], F32, kind="ExternalInput")
    ones_d = nc.dram_tensor("ones_mt", [P, P], DTW, kind="ExternalInput")
    out_d = nc.dram_tensor("out", [C, NQ], F32, kind="ExternalOutput")

    with tile.TileContext(nc) as tc:
        with (
            tc.tile_pool(name="const", bufs=1) as pc,
            tc.tile_pool(name="xh", bufs=1) as pxh,
            tc.tile_pool(name="wp", bufs=8) as pw,
            tc.tile_pool(name="qx", bufs=8) as pqx,
            tc.tile_pool(name="attn", bufs=1) as pat,
        ):
            # ---- loads (x halves first: per-queue FIFO prioritizes x) --
            xs = []
            for t in range(CT):
                xt = pxh.tile([P, HW], DTX, tag=f"x{t}")
                for hh in range(2):
                    nc.sync.dma_start(
                        xt[:, hh * 2048:(hh + 1) * 2048],
                        x_d[t * P:(t + 1) * P, hh * 2048:(hh + 1) * 2048])
                xs.append(xt)

            def load_w(dram):
                ts = []
                for ci in range(CT):
                    wt = pw.tile([P, C], DTW, tag="w")
                    nc.sync.dma_start(wt[:], dram[ci * P:(ci + 1) * P, :])
                    ts.append(wt)
                return ts

            wq = load_w(wq_d)
            wk = load_w(wk_d)
            wv = load_w(wv_d)

            bq_t = pc.tile([P, CT], F32, tag="bq")
            bo_t = pc.tile([P, CT], F32, tag="bo")
            gns_t = pc.tile([P, CT], F32, tag="gns")
            gnb_t = pc.tile([P, CT], F32, tag="gnb")
            g1_t = pc.tile([P, 8], F32, tag="g1")
            g2_t = pc.tile([8, P], F32, tag="g2")
            ones_t = pc.tile([P, P], DTW, tag="ones")
            for tl, dr in ((bq_t, bq_d), (bo_t, bo_d),
                           (gns_t, gns_d), (gnb_t, gnb_d), (g1_t, g1_d),
                           (g2_t, g2_d), (ones_t, ones_d)):
                nc.sync.dma_start(tl[:], dr[:])

            # ---- phase A: group norm, phase B: q/k projections ----------
            with tc.tile_pool(name="ps1", bufs=1, space="PSUM") as ps1:
                hs = []
                for t in range(CT):
                    xt = xs[t]
                    bn6 = pc.tile([P, 8, 6], F32, tag="bn6", bufs=2)
                    for r in range(8):
                        nc.vector.bn_stats(bn6[:, r, :],
                                           _xf(xt[:, r * 512:(r + 1) * 512]))
                    bn2 = pc.tile([P, 2], F32, tag=f"bn2_{t}")
                    nc.vector.bn_aggr(bn2[:], bn6[:])
                    # stats2 = [mean, E[x^2]] per partition
                    st2 = pc.tile([P, 2], F32, tag=f"st2_{t}")
                    nc.scalar.activation(st2[:, 0:1], bn2[:, 0:1], AF.Copy)
                    msq = pc.tile([P, 1], F32, tag=f"msq_{t}")
                    nc.scalar.activation(msq[:], bn2[:, 0:1], AF.Square)
                    nc.scalar.activation(st2[:, 1:2], msq[:], AF.Identity,
                                         bias=bn2[:, 1:2])
                    # group-combine (8 groups of 16 partitions): [8, 2]
                    psg = ps1.tile([8, 2], F32, tag="psg")
                    nc.tensor.matmul(psg[:], g1_t[:], st2[:],
                                     start=True, stop=True)
                    sg = pc.tile([8, 2], F32, tag=f"sg_{t}")
                    nc.scalar.activation(sg[:], psg[:], AF.Copy)
                    # broadcast back to 128 partitions
                    psb = ps1.tile([P, 2], F32, tag="psb")
                    nc.tensor.matmul(psb[:], g2_t[:], sg[:],
                                     start=True, stop=True)
                    sbc = pc.tile([P, 2], F32, tag=f"sbc_{t}")
                    nc.scalar.activation(sbc[:], psb[:], AF.Copy)
                    mean_c = sbc[:, 0:1]
                    ex2_c = sbc[:, 1:2]
                    msq2 = pc.tile([P, 1], F32, tag=f"msq2_{t}")
                    nc.scalar.activation(msq2[:], mean_c, AF.Square)
                    negm = pc.tile([P, 1], F32, tag=f"negm_{t}")
                    nc.vector.tensor_scalar(negm[:], msq2[:], -1.0, EPS,
                                            op0=ALU.mult, op1=ALU.add)
                    std = pc.tile([P, 1], F32, tag=f"std_{t}")
                    nc.scalar.activation(std[:], ex2_c, AF.Sqrt, bias=negm[:])
                    inv = pc.tile([P, 1], F32, tag=f"inv_{t}")
                    nc.vector.reciprocal(inv[:], std[:])
                    a_t = pc.tile([P, 1], F32, tag=f"a_{t}")
                    nc.scalar.activation(a_t[:], inv[:], AF.Identity,
                                         scale=gns_t[:, t:t + 1])
                    am = pc.tile([P, 1], F32, tag=f"am_{t}")
                    nc.scalar.activation(am[:], mean_c, AF.Identity,
                                         scale=a_t[:])
                    bvec = pc.tile([P, 1], F32, tag=f"bv_{t}")
                    nc.scalar.activation(bvec[:], am[:], AF.Identity,
                                         scale=-1.0, bias=gnb_t[:, t:t + 1])
                    # h = a * x + b   (in-place over x for f32r mode)
                    if MODE == "f32r":
                        ht = xt
                    else:
                        ht = pxh.tile([P, HW], BF16, tag=f"h{t}")
                    nc.scalar.activation(ht[:, 0:2048], _xf(xt[:, 0:2048]),
                                         AF.Identity, bias=bvec[:],
                                         scale=a_t[:])
                    nc.scalar.activation(ht[:, 2048:HW], _xf(xt[:, 2048:HW]),
                                         AF.Identity, bias=bvec[:],
                                         scale=a_t[:])
                    hs.append(ht)

                def h_ap(t):
                    return hs[t][:]

                # ---- q [C, NQ] and k [C, HW] ---------------------------
                qs = []
                for co in range(CT):
                    qt = pqx.tile([P, NQ], DTT, tag="qx")
                    for nh in range(NT):
                        pq = ps1.tile([P, 512], F32, tag="pqk", bufs=2)
                        for ci in range(CT):
                            nc.tensor.matmul(
                                pq[:],
                                _rc(wq[ci][:, co * P:(co + 1) * P]),
                                h_ap(ci)[:, nh * 512:(nh + 1) * 512],
                                start=(ci == 0), stop=(ci == CT - 1))
                        nc.scalar.activation(_wc(qt[:, nh * 512:(nh + 1) * 512]),
                                             pq[:], AF.Identity,
                                             bias=bq_t[:, co:co + 1])
                    qs.append(qt)

                # qk[ci, n] = sum_co wk[co, ci] q[co, n]  (K folded away)
                qks = []
                for ci in range(CT):
                    qkt = pqx.tile([P, NQ], DTT, tag="qx", name=f"qk{ci}")
                    for nh in range(NT):
                        pqk2 = ps1.tile([P, 512], F32, tag="pqk", bufs=2)
                        for co in range(CT):
                            nc.tensor.matmul(
                                pqk2[:],
                                _rc(wk[co][:, ci * P:(ci + 1) * P]),
                                _rc(qs[co][:, nh * 512:(nh + 1) * 512]),
                                start=(co == 0), stop=(co == CT - 1))
                        nc.scalar.activation(
                            _wc(qkt[:, nh * 512:(nh + 1) * 512]),
                            pqk2[:], AF.Copy)
                    qks.append(qkt)

                # vT precompute: vT[m, c] = sum_ci h[ci, m] wvT[ci, c]
                vts = []
                for mi in range(MT):
                    msl = slice(mi * P, (mi + 1) * P)
                    pv = ps1.tile([P, 512], F32, tag="pv", bufs=2)
                    for ci in range(CT):
                        nc.tensor.matmul(pv[:], h_ap(ci)[:, msl],
                                         _rc(wv[ci][:]),
                                         start=(ci == 0), stop=(ci == CT - 1))
                    vt = pat.tile([P, 512], DTT, tag="vt", bufs=MT,
                                  name=f"vt{mi}")
                    nc.vector.tensor_copy(_wc(vt[:]), pv[:])
                    vts.append(vt)

            wo = load_w(wo_d)   # reuses wq's pool slots (wq dead after q)

            # residual prefetch (qx slots free once qk has consumed q)
            xres = []
            for co in range(CT):
                xr = pqx.tile([P, NQ], F32, tag="qx", name=f"xres{co}")
                nc.sync.dma_start(xr[:], _xf(x_d[co * P:(co + 1) * P, 0:NQ]))
                nc.scalar.activation(xr[:], xr[:], AF.Identity,
                                     bias=bo_t[:, co:co + 1])
                xres.append(xr)

            # ---- phase C: attention ------------------------------------
            accs_all = []
            with tc.tile_pool(name="ps2", bufs=1, space="PSUM") as ps2:
                for nt in range(NT):
                    nsl = slice(nt * 512, (nt + 1) * 512)
                    po = [ps2.tile([P, 512], F32, tag="po", bufs=4,
                                   name=f"po{nt}_{_cj}")
                          for _cj in range(CT)]
                    pden = ps2.tile([P, 512], F32, tag="pden")
                    for mi in range(MT):
                        msl = slice(mi * P, (mi + 1) * P)
                        # sT [128m, 512n] = sum_ci h[ci, m] qk[ci, n]
                        pst = ps2.tile([P, 512], F32, tag="pst", bufs=3)
                        for ci in range(CT):
                            nc.tensor.matmul(pst[:], h_ap(ci)[:, msl],
                                             _rc(qks[ci][:, nsl]),
                                             start=(ci == 0), stop=(ci == CT - 1))
                        et = pat.tile([P, 512], DTT, tag="et", bufs=3)
                        nc.scalar.activation(_wc(et[:]), pst[:], AF.Exp)
                        # denominator accumulation
                        nc.tensor.matmul(pden[:], _rc(ones_t[:]), _rc(et[:]),
                                         start=(mi == 0), stop=(mi == MT - 1))
                        # attn @ v accumulation
                        for cj in range(CT):
                            nc.tensor.matmul(
                                po[cj][:],
                                _rc(vts[mi][:, cj * P:(cj + 1) * P]),
                                _rc(et[:]),
                                start=(mi == 0), stop=(mi == MT - 1))
                    # normalize: out = po * (1/denom), denom broadcast
                    rbc = pat.tile([P, 512], F32, tag="rbc", bufs=2)
                    nc.vector.reciprocal(rbc[:], pden[:])
                    accs = []
                    for cj in range(CT):
                        acc = pat.tile([P, 512], DTT, tag="acc", bufs=4,
                                       name=f"acc{nt}_{cj}")
                        nc.vector.tensor_tensor(_wc(acc[:]), po[cj][:], rbc[:],
                                                op=ALU.mult)
                        accs.append(acc)
                    # output projection + residual for this nt
                    for co in range(CT):
                        pp = ps2.tile([P, 512], F32, tag="po", bufs=4)
                        for ci in range(CT):
                            nc.tensor.matmul(
                                pp[:],
                                _rc(wo[ci][:, co * P:(co + 1) * P]),
                                _rc(accs[ci][:]),
                                start=(ci == 0), stop=(ci == CT - 1))
                        nc.vector.tensor_tensor(xres[co][:, nsl], pp[:],
                                                xres[co][:, nsl], op=ALU.add)
                for co in range(CT):
                    nc.sync.dma_start(out_d[co * P:(co + 1) * P, :], xres[co][:])
    nc.compile()
    return nc


_NC_CACHE = {}


def _get_nc():
    if "nc" not in _NC_CACHE:
        _NC_CACHE["nc"] = build_nc()
    return _NC_CACHE["nc"]


def _host_arrays(inputs):
    f = lambda a: np.asarray(a, dtype=np.float32)
    x = f(inputs["x"]).reshape(B, C, HW)
    scale = np.float32(C) ** np.float32(-0.5)
    wq, bq = f(inputs["wq"]), f(inputs["bq"])
    wk, bk = f(inputs["wk"]), f(inputs["bk"])
    wv, bv = f(inputs["wv"]), f(inputs["bv"])
    wo, bo = f(inputs["wo"]), f(inputs["bo"])

    if MODE == "f32r":
        cw = lambda a: np.ascontiguousarray(a, dtype=np.float32)
    else:
        import ml_dtypes
        cw = lambda a: np.ascontiguousarray(a).astype(ml_dtypes.bfloat16)

    vec = lambda v: np.ascontiguousarray(
        np.asarray(v, np.float32).reshape(CT, P).T)

    g1 = np.zeros((P, 8), np.float32)
    g2 = np.zeros((8, P), np.float32)
    for p in range(P):
        g1[p, p // GS] = 1.0 / GS
        g2[p // GS, p] = 1.0
    shared = {
        "wqt": cw(wq.T * scale),
        "wkp": cw(wk),
        "wvt": cw(wv.T),
        "wot": cw(wo.T),
        "bq2": vec(bq * scale),
        "bo2": vec((wo.astype(np.float64) @ bv.astype(np.float64)
                    + bo).astype(np.float32)),
        "gns": vec(f(inputs["gn_scale"])),
        "gnb": vec(f(inputs["gn_bias"])),
        "g1": g1,
        "g2": g2,
        "ones_mt": cw(np.ones((P, P), np.float32)),
    }
    in_maps = []
    for core in range(8):
        b, j = divmod(core, 4)
        xr = np.roll(x[b], -NQ * j, axis=1) if j else x[b]
        im = dict(shared)
        im["x"] = np.ascontiguousarray(xr)
        in_maps.append(im)
    return in_maps


def _gather(results):
    y = np.empty((B, C, HW), np.float32)
    for core in range(8):
        b, j = divmod(core, 4)
        y[b][:, NQ * j:NQ * (j + 1)] = results[core]["out"]
    return y.reshape(B, C, H, W)


def _run(inputs, trace=False, tmpdir=None):
    nc = _get_nc()
    in_maps = _host_arrays(inputs)
    res = run_bass_kernel_spmd(nc, in_maps, list(range(8)), trace=trace,
                               tmpdir=tmpdir)
    return _gather(res.results), res


def kernel(**inputs):
    out, _ = _run(inputs)
    return out


# revision 15
# speedup vs baseline: 1.0281x; 1.0281x over previous
"""Trainium2 Bass kernel for nn_AttnBlock (GroupNorm + single-head self-attention + residual).

Full-input contract: kernel(**inputs) takes the unsharded inputs from
setup_inputs() and returns the full [2, 512, 64, 64] output.

Sharding: 8 cores = 2 batches x 4 query-token chunks of 1024 tokens.
Each core receives its batch's x with tokens rotated so its query chunk
is columns 0:1024 (attention / groupnorm are permutation-invariant over
tokens).  Each core computes GroupNorm + full K / V-transpose for its
batch (duplicated across the 4 cores of a batch -- no collectives), and
the attention rows + output projection + residual for its own 1024
queries.

Attention is computed in transposed-score form to avoid all on-chip
transposes:
    sT[m, n]   = sum_c k[c, m] q[c, n]          (lhsT = k, rhs = q)
    vT[m, c]   = sum_ci h[ci, m] wvT[ci, c]     (lhsT = h, rhs = wvT)
    raw[c, n]  = sum_m vT[m, c] expT[m, n]      (lhsT = vT, rhs = expT)
    out[c, n]  = raw[c, n] / denom[n]
The softmax max-subtraction is dropped (scores are O(1) here; exp stays
comfortably in fp32 range and softmax is shift-invariant).  bv folds
into the output-projection bias (softmax rows sum to 1); bk is applied
normally.
"""

import numpy as np

import concourse.bass as bass
import concourse.bacc as bacc
import concourse.mybir as mybir
import concourse.tile as tile
from concourse.bass_utils import run_bass_kernel_spmd

AF = mybir.ActivationFunctionType
ALU = mybir.AluOpType

# Problem shape (hardcoded per contract).
B, C, H, W = 2, 512, 64, 64
HW = H * W            # 4096 tokens
GROUPS, GS = 32, 16   # 16 channels per group
EPS = 1e-6
P = 128               # partitions
CT = C // P           # 4 channel tiles
NQ = HW // 4          # 1024 query tokens per core
NT = NQ // 512        # 2 n-tiles of 512
MT = HW // P          # 32 m-tiles of 128

MODE = "f32r"         # "f32r" | "bf16" matmul operand mode

F32 = mybir.dt.float32
F32R = mybir.dt.float32r
BF16 = mybir.dt.bfloat16
DTW = F32R if MODE == "f32r" else BF16      # weight dram/tile dtype
DTX = F32R if MODE == "f32r" else F32        # x dram/tile dtype (in-place h)
DTT = F32 if MODE == "f32r" else BF16       # mm-feeding computed-tile dtype


def _wc(ap):
    """Cast an output AP that feeds a matmul (producer must round)."""
    return ap.bitcast(F32R) if MODE == "f32r" else ap


def _rc(ap):
    """Cast a matmul operand AP (f32-stored tiles read as f32r)."""
    return ap.bitcast(F32R) if MODE == "f32r" else ap


def _xf(ap):
    """f32 view of an x tile (typed f32r in f32r mode) for non-matmul readers."""
    return ap.bitcast(F32) if MODE == "f32r" else ap


def build_nc():
    nc = bacc.Bacc()

    x_d = nc.dram_tensor("x", [C, HW], DTX, kind="ExternalInput")
    wq_d = nc.dram_tensor("wqt", [C, C], DTW, kind="ExternalInput")
    wk_d = nc.dram_tensor("wkp", [C, C], DTW, kind="ExternalInput")
    wv_d = nc.dram_tensor("wvt", [C, C], DTW, kind="ExternalInput")
    wo_d = nc.dram_tensor("wot", [C, C], DTW, kind="ExternalInput")
    # [128, 4] per-partition vectors (col = channel-tile index)
    bq_d = nc.dram_tensor("bq2", [P, CT], F32, kind="ExternalInput")
    bo_d = nc.dram_tensor("bo2", [P, CT], F32, kind="ExternalInput")
    gns_d = nc.dram_tensor("gns", [P, CT], F32, kind="ExternalInput")
    gnb_d = nc.dram_tensor("gnb", [P, CT], F32, kind="ExternalInput")
    g1_d = nc.dram_tensor("g1", [P, 8], F32, kind="ExternalInput")
    g2_d = nc.dram_tensor("g2", [8, P], F32, kind="ExternalInput")
    ones_d = nc.dram_tensor("ones_mt", [P, P], DTW, kind="ExternalInput")
    out_d = nc.dram_tensor("out", [C, NQ], F32, kind="ExternalOutput")

    with tile.TileContext(nc) as tc:
        with (
            tc.tile_pool(name="const", bufs=1) as pc,
            tc.tile_pool(name="xh", bufs=1) as pxh,
            tc.tile_pool(name="wp", bufs=8) as pw,
            tc.tile_pool(name="qx", bufs=8) as pqx,
            tc.tile_pool(name="attn", bufs=1) as pat,
        ):
            # ---- loads (x halves first: per-queue FIFO prioritizes x) --
            xs = []
            for t in range(CT):
                xt = pxh.tile([P, HW], DTX, tag=f"x{t}")
                for hh in range(2):
                    nc.sync.dma_start(
                        xt[:, hh * 2048:(hh + 1) * 2048],
                        x_d[t * P:(t + 1) * P, hh * 2048:(hh + 1) * 2048])
                xs.append(xt)

            def load_w(dram):
                ts = []
                for ci in range(CT):
                    wt = pw.tile([P, C], DTW, tag="w")
                    nc.sync.dma_start(wt[:], dram[ci * P:(ci + 1) * P, :])
                    ts.append(wt)
                return ts

            wq = load_w(wq_d)
            wk = load_w(wk_d)
            wv = load_w(wv_d)

            bq_t = pc.tile([P, CT], F32, tag="bq")
            bo_t = pc.tile([P, CT], F32, tag="bo")
            gns_t = pc.tile([P, CT], F32, tag="gns")
            gnb_t = pc.tile([P, CT], F32, tag="gnb")
            g1_t = pc.tile([P, 8], F32, tag="g1")
            g2_t = pc.tile([8, P], F32, tag="g2")
            ones_t = pc.tile([P, P], DTW, tag="ones")
            for tl, dr in ((bq_t, bq_d), (bo_t, bo_d),
                           (gns_t, gns_d), (gnb_t, gnb_d), (g1_t, g1_d),
                           (g2_t, g2_d), (ones_t, ones_d)):
                nc.sync.dma_start(tl[:], dr[:])

            # ---- phase A: group norm, phase B: q/k projections ----------
            with tc.tile_pool(name="ps1", bufs=1, space="PSUM") as ps1:
                hs = []
                for t in range(CT):
                    xt = xs[t]
                    bn6 = pc.tile([P, 8, 6], F32, tag="bn6", bufs=2)
                    for r in range(8):
                        nc.vector.bn_stats(bn6[:, r, :],
                                           _xf(xt[:, r * 512:(r + 1) * 512]))
                    bn2 = pc.tile([P, 2], F32, tag=f"bn2_{t}")
                    nc.vector.bn_aggr(bn2[:], bn6[:])
                    # stats2 = [mean, E[x^2]] per partition
                    st2 = pc.tile([P, 2], F32, tag=f"st2_{t}")
                    nc.vector.tensor_copy(st2[:, 0:1], bn2[:, 0:1])
                    msq = pc.tile([P, 1], F32, tag=f"msq_{t}")
                    nc.vector.tensor_tensor(msq[:], bn2[:, 0:1], bn2[:, 0:1],
                                            op=ALU.mult)
                    nc.vector.tensor_tensor(st2[:, 1:2], bn2[:, 1:2],
                                            msq[:], op=ALU.add)
                    # group-combine (8 groups of 16 partitions): [8, 2]
                    psg = ps1.tile([8, 2], F32, tag="psg")
                    nc.tensor.matmul(psg[:], g1_t[:], st2[:],
                                     start=True, stop=True)
                    sg = pc.tile([8, 2], F32, tag=f"sg_{t}")
                    nc.scalar.activation(sg[:], psg[:], AF.Copy)
                    # broadcast back to 128 partitions
                    psb = ps1.tile([P, 2], F32, tag="psb")
                    nc.tensor.matmul(psb[:], g2_t[:], sg[:],
                                     start=True, stop=True)
                    sbc = pc.tile([P, 2], F32, tag=f"sbc_{t}")
                    nc.scalar.activation(sbc[:], psb[:], AF.Copy)
                    mean_c = sbc[:, 0:1]
                    ex2_c = sbc[:, 1:2]
                    msq2 = pc.tile([P, 1], F32, tag=f"msq2_{t}")
                    nc.vector.tensor_tensor(msq2[:], mean_c, mean_c, op=ALU.mult)
                    negm = pc.tile([P, 1], F32, tag=f"negm_{t}")
                    nc.vector.tensor_scalar(negm[:], msq2[:], -1.0, EPS,
                                            op0=ALU.mult, op1=ALU.add)
                    std = pc.tile([P, 1], F32, tag=f"std_{t}")
                    nc.scalar.activation(std[:], ex2_c, AF.Sqrt, bias=negm[:])
                    inv = pc.tile([P, 1], F32, tag=f"inv_{t}")
                    nc.vector.reciprocal(inv[:], std[:])
                    a_t = pc.tile([P, 1], F32, tag=f"a_{t}")
                    nc.vector.tensor_tensor(a_t[:], inv[:], gns_t[:, t:t + 1],
                                            op=ALU.mult)
                    am = pc.tile([P, 1], F32, tag=f"am_{t}")
                    nc.vector.tensor_tensor(am[:], a_t[:], mean_c, op=ALU.mult)
                    bvec = pc.tile([P, 1], F32, tag=f"bv_{t}")
                    nc.vector.tensor_tensor(bvec[:], gnb_t[:, t:t + 1], am[:],
                                            op=ALU.subtract)
                    # h = a * x + b   (in-place over x for f32r mode)
                    if MODE == "f32r":
                        ht = xt
                    else:
                        ht = pxh.tile([P, HW], BF16, tag=f"h{t}")
                    nc.scalar.activation(ht[:, 0:2048], _xf(xt[:, 0:2048]),
                                         AF.Identity, bias=bvec[:],
                                         scale=a_t[:])
                    nc.vector.tensor_scalar(ht[:, 2048:HW],
                                            _xf(xt[:, 2048:HW]),
                                            a_t[:], bvec[:],
                                            op0=ALU.mult, op1=ALU.add)
                    hs.append(ht)

                def h_ap(t):
                    return hs[t][:]

                # ---- q [C, NQ] and k [C, HW] ---------------------------
                qs = []
                for co in range(CT):
                    qt = pqx.tile([P, NQ], DTT, tag="qx")
                    for nh in range(NT):
                        pq = ps1.tile([P, 512], F32, tag="pqk", bufs=2)
                        for ci in range(CT):
                            nc.tensor.matmul(
                                pq[:],
                                _rc(wq[ci][:, co * P:(co + 1) * P]),
                                h_ap(ci)[:, nh * 512:(nh + 1) * 512],
                                start=(ci == 0), stop=(ci == CT - 1))
                        nc.scalar.activation(_wc(qt[:, nh * 512:(nh + 1) * 512]),
                                             pq[:], AF.Identity,
                                             bias=bq_t[:, co:co + 1])
                    qs.append(qt)

                # qk[ci, n] = sum_co wk[co, ci] q[co, n]  (K folded away)
                qks = []
                for ci in range(CT):
                    qkt = pqx.tile([P, NQ], DTT, tag="qx", name=f"qk{ci}")
                    for nh in range(NT):
                        pqk2 = ps1.tile([P, 512], F32, tag="pqk", bufs=2)
                        for co in range(CT):
                            nc.tensor.matmul(
                                pqk2[:],
                                _rc(wk[co][:, ci * P:(ci + 1) * P]),
                                _rc(qs[co][:, nh * 512:(nh + 1) * 512]),
                                start=(co == 0), stop=(co == CT - 1))
                        nc.scalar.activation(
                            _wc(qkt[:, nh * 512:(nh + 1) * 512]),
                            pqk2[:], AF.Copy)
                    qks.append(qkt)

                # vT precompute: vT[m, c] = sum_ci h[ci, m] wvT[ci, c]
                vts = []
                for mi in range(MT):
                    msl = slice(mi * P, (mi + 1) * P)
                    pv = ps1.tile([P, 512], F32, tag="pv", bufs=2)
                    for ci in range(CT):
                        nc.tensor.matmul(pv[:], h_ap(ci)[:, msl],
                                         _rc(wv[ci][:]),
                                         start=(ci == 0), stop=(ci == CT - 1))
                    vt = pat.tile([P, 512], DTT, tag="vt", bufs=MT,
                                  name=f"vt{mi}")
                    nc.vector.tensor_copy(_wc(vt[:]), pv[:])
                    vts.append(vt)

            wo = load_w(wo_d)   # reuses wq's pool slots (wq dead after q)

            # residual prefetch (qx slots free once qk has consumed q)
            xres = []
            for co in range(CT):
                xr = pqx.tile([P, NQ], F32, tag="qx", name=f"xres{co}")
                nc.sync.dma_start(xr[:], _xf(x_d[co * P:(co + 1) * P, 0:NQ]))
                nc.scalar.activation(xr[:], xr[:], AF.Identity,
                                     bias=bo_t[:, co:co + 1])
                xres.append(xr)

            # ---- phase C: attention ------------------------------------
            accs_all = []
            with tc.tile_pool(name="ps2", bufs=1, space="PSUM") as ps2:
                for nt in range(NT):
                    nsl = slice(nt * 512, (nt + 1) * 512)
                    po = [ps2.tile([P, 512], F32, tag="po", bufs=4,
                                   name=f"po{nt}_{_cj}")
                          for _cj in range(CT)]
                    pden = ps2.tile([P, 512], F32, tag="pden")
                    for mi in range(MT):
                        msl = slice(mi * P, (mi + 1) * P)
                        # sT [128m, 512n] = sum_ci h[ci, m] qk[ci, n]
                        pst = ps2.tile([P, 512], F32, tag="pst", bufs=2)
                        for ci in range(CT):
                            nc.tensor.matmul(pst[:], h_ap(ci)[:, msl],
                                             _rc(qks[ci][:, nsl]),
                                             start=(ci == 0), stop=(ci == CT - 1))
                        et = pat.tile([P, 512], DTT, tag="et", bufs=2)
                        nc.scalar.activation(_wc(et[:]), pst[:], AF.Exp)
                        # denominator accumulation
                        nc.tensor.matmul(pden[:], _rc(ones_t[:]), _rc(et[:]),
                                         start=(mi == 0), stop=(mi == MT - 1))
                        # attn @ v accumulation
                        for cj in range(CT):
                            nc.tensor.matmul(
                                po[cj][:],
                                _rc(vts[mi][:, cj * P:(cj + 1) * P]),
                                _rc(et[:]),
                                start=(mi == 0), stop=(mi == MT - 1))
                    # normalize: out = po * (1/denom), denom broadcast
                    rbc = pat.tile([P, 512], F32, tag="rbc", bufs=2)
                    nc.vector.reciprocal(rbc[:], pden[:])
                    accs = []
                    for cj in range(CT):
                        acc = pat.tile([P, 512], DTT, tag="acc", bufs=4,
                                       name=f"acc{nt}_{cj}")
                        nc.vector.tensor_tensor(_wc(acc[:]), po[cj][:], rbc[:],
                                                op=ALU.mult)
                        accs.append(acc)
                    # output projection + residual for this nt
                    for co in range(CT):
                        pp = ps2.tile([P, 512], F32, tag="po", bufs=4)
                        for ci in range(CT):
                            nc.tensor.matmul(
                                pp[:],
                                _rc(wo[ci][:, co * P:(co + 1) * P]),
                                _rc(accs[ci][:]),
                                start=(ci == 0), stop=(ci == CT - 1))
                        nc.vector.tensor_tensor(xres[co][:, nsl], pp[:],
                                                xres[co][:, nsl], op=ALU.add)
                for co in range(CT):
                    nc.sync.dma_start(out_d[co * P:(co + 1) * P, :], xres[co][:])
    nc.compile()
    return nc


_NC_CACHE = {}


def _get_nc():
    if "nc" not in _NC_CACHE:
        _NC_CACHE["nc"] = build_nc()
    return _NC_CACHE["nc"]


def _host_arrays(inputs):
    f = lambda a: np.asarray(a, dtype=np.float32)
    x = f(inputs["x"]).reshape(B, C, HW)
    scale = np.float32(C) ** np.float32(-0.5)
    wq, bq = f(inputs["wq"]), f(inputs["bq"])
    wk, bk = f(inputs["wk"]), f(inputs["bk"])
    wv, bv = f(inputs["wv"]), f(inputs["bv"])
    wo, bo = f(inputs["wo"]), f(inputs["bo"])

    if MODE == "f32r":
        cw = lambda a: np.ascontiguousarray(a, dtype=np.float32)
    else:
        import ml_dtypes
        cw = lambda a: np.ascontiguousarray(a).astype(ml_dtypes.bfloat16)

    vec = lambda v: np.ascontiguousarray(
        np.asarray(v, np.float32).reshape(CT, P).T)

    g1 = np.zeros((P, 8), np.float32)
    g2 = np.zeros((8, P), np.float32)
    for p in range(P):
        g1[p, p // GS] = 1.0 / GS
        g2[p // GS, p] = 1.0
    shared = {
        "wqt": cw(wq.T * scale),
        "wkp": cw(wk),
        "wvt": cw(wv.T),
        "wot": cw(wo.T),
        "bq2": vec(bq * scale),
        "bo2": vec((wo.astype(np.float64) @ bv.astype(np.float64)
                    + bo).astype(np.float32)),
        "gns": vec(f(inputs["gn_scale"])),
        "gnb": vec(f(inputs["gn_bias"])),
        "g1": g1,
        "g2": g2,
        "ones_mt": cw(np.ones((P, P), np.float32)),
    }
    in_maps = []
    for core in range(8):
        b, j = divmod(core, 4)
        xr = np.roll(x[b], -NQ * j, axis=1) if j else x[b]
        im = dict(shared)
        im["x"] = np.ascontiguousarray(xr)
        in_maps.append(im)
    return in_maps


def _gather(results):
    y = np.empty((B, C, HW), np.float32)
    for core in range(8):
        b, j = divmod(core, 4)
        y[b][:, NQ * j:NQ * (j + 1)] = results[core]["out"]
    return y.reshape(B, C, H, W)


def _run(inputs, trace=False, tmpdir=None):
    nc = _get_nc()
    in_maps = _host_arrays(inputs)
    res = run_bass_kernel_spmd(nc, in_maps, list(range(8)), trace=trace,
                               tmpdir=tmpdir)
    return _gather(res.results), res


def kernel(**inputs):
    out, _ = _run(inputs)
    return out
